# revision 6
# baseline (speedup 1.0000x reference)
"""Bass/Trainium2 kernel for nn_EntangledInterferenceLayer (8 NeuronCores).

Sharding: DP over batch (4) x TP over heads (2 groups of 8) = 8 cores.
Core c handles batch b = c >> 1, head group g = c & 1.
Each core returns a partial out-projection (contracting its 512 attention
dims); the host adds the two partials per batch (+ output bias).

Host-side exact transformations:
- Entanglement einsum folded into the Q/K weight matrices (rope commutes
  with the head-mixing einsum, so this is exact).
- Attention scale 1/sqrt(64) folded into the Q weights.
- Per-head dims de-interleaved (rope pairs (2j,2j+1) -> (j, 16+j)) so rope
  becomes contiguous-block ops; attention is invariant to this perm.
- softmax computed as exp(c*sqrt(m+eps))/rowsum (logits small, no max-sub);
  sqrt via exp(0.5*ln(.)) so all ACT functions live in one table set.
- All matmul operands are float32r (11-bit mantissa, 1 cyc/row at N>=256);
  host pre-rounds DMA'd values onto the f32r grid.
"""
import sys

sys.path.insert(0, '/opt/trn_rl_repo')

import numpy as np
from contextlib import ExitStack

import concourse.bass as bass
from concourse import bacc
import concourse.tile as tile
from concourse import mybir
from concourse.bass_utils import run_bass_kernel_spmd

F32 = mybir.dt.float32
F32R = mybir.dt.float32r
AF = mybir.ActivationFunctionType
AX = mybir.AxisListType

B, S, D, H = 4, 1024, 1024, 16
HD = 64
NJ = 16                  # rotation pairs (ROT=32)
N_CORES = 8
HPC = H // 2             # heads per core = 8
ST = S // 128            # s-tiles = 8
KC = D // 128            # contraction chunks = 8


def round_f32r(x: np.ndarray) -> np.ndarray:
    """Round fp32 to the f32r grid (11-bit mantissa, RNE)."""
    b = np.ascontiguousarray(x.astype(np.float32)).view(np.uint32)
    lsb = (b >> np.uint32(12)) & np.uint32(1)
    b = b + np.uint32(0x7FF) + lsb
    b = b & np.uint32(0xFFFFF000)
    return b.view(np.float32)


def _chunks_for_qtile(t):
    """k-chunks [(pos, width, valid_w)] for q-tile t; valid k < 128*(t+1).
    Widths >= 256 (f32r needs N>=256); the tail chunk may be padded."""
    kw = 128 * (t + 1)
    out = []
    pos = 0
    while kw - pos >= 512:
        out.append((pos, 512, 512))
        pos += 512
    rem = kw - pos
    if rem > 0:
        out.append((pos, max(256, rem), rem))
    return out


def build_program():
    nc = bacc.Bacc("TRN2", target_bir_lowering=False, debug=False,
                   num_devices=N_CORES)

    XR = nc.dram_tensor("XR", [S, D], F32R, kind="ExternalInput").ap()
    XI = nc.dram_tensor("XI", [S, D], F32R, kind="ExternalInput").ap()
    WQR = nc.dram_tensor("WQR", [D, HPC * HD], F32R, kind="ExternalInput").ap()
    WQI = nc.dram_tensor("WQI", [D, HPC * HD], F32R, kind="ExternalInput").ap()
    WKR = nc.dram_tensor("WKR", [D, HPC * HD], F32R, kind="ExternalInput").ap()
    WKI = nc.dram_tensor("WKI", [D, HPC * HD], F32R, kind="ExternalInput").ap()
    WVR = nc.dram_tensor("WVR", [D, HPC * HD], F32R, kind="ExternalInput").ap()
    WVI = nc.dram_tensor("WVI", [D, HPC * HD], F32R, kind="ExternalInput").ap()
    WOR = nc.dram_tensor("WOR", [HPC * HD, D], F32R, kind="ExternalInput").ap()
    WOI = nc.dram_tensor("WOI", [HPC * HD, D], F32R, kind="ExternalInput").ap()
    IDN = nc.dram_tensor("IDN", [128, 128], F32R, kind="ExternalInput").ap()
    # rope tables, layout (t, h4, j) so ops match psum (h,d) APs directly
    CT = nc.dram_tensor("CT", [128, ST * 4 * NJ], F32, kind="ExternalInput").ap()
    STB = nc.dram_tensor("STB", [128, ST * 4 * NJ], F32, kind="ExternalInput").ap()
    # phase tables (std layout, equal rows): [128, HPC*64]
    TPC = nc.dram_tensor("TPC", [128, HPC * HD], F32, kind="ExternalInput").ap()
    TPS = nc.dram_tensor("TPS", [128, HPC * HD], F32, kind="ExternalInput").ap()
    TRI = nc.dram_tensor("TRI", [128, 128], F32, kind="ExternalInput").ap()
    CC = nc.dram_tensor("CC", [128, 2], F32, kind="ExternalInput").ap()
    OUTR = nc.dram_tensor("OUTR", [S, D], F32, kind="ExternalOutput").ap()
    OUTI = nc.dram_tensor("OUTI", [S, D], F32, kind="ExternalOutput").ap()

    with tile.TileContext(nc) as tc, ExitStack() as ctx:
        consts = ctx.enter_context(tc.tile_pool(name="consts", bufs=1))
        ident = consts.tile([128, 128], F32R)
        nc.sync.dma_start(ident[:], IDN)
        ct = consts.tile([128, ST * 4 * NJ], F32)
        stb = consts.tile([128, ST * 4 * NJ], F32)
        nc.sync.dma_start(ct[:], CT)
        nc.sync.dma_start(stb[:], STB)
        tpc = consts.tile([128, HPC * HD], F32)
        tpsn = consts.tile([128, HPC * HD], F32)
        nc.sync.dma_start(tpc[:], TPC)
        nc.sync.dma_start(tpsn[:], TPS)
        tri = consts.tile([128, 128], F32)
        nc.sync.dma_start(tri[:], TRI)
        cc = consts.tile([128, 2], F32)
        nc.sync.dma_start(cc[:], CC)
        epsc = cc[:, 0:1]
        lncc = cc[:, 1:2]

        # attn output (transposed, f32r): [din-chunk][128, S] for r and i.
        # Partition layout per chunk d: attn_r = [r(h=2d) | r(h=2d+1)],
        # attn_i = [i(h=2d+1) | i(h=2d)]  (host permutes WOI rows to match).
        attnp = ctx.enter_context(tc.tile_pool(name="attnp", bufs=1))
        attn_r = [attnp.tile([128, S], F32R, tag=f"atr{k}", name=f"attn_r{k}")
                  for k in range(4)]
        attn_i = [attnp.tile([128, S], F32R, tag=f"ati{k}", name=f"attn_i{k}")
                  for k in range(4)]

        vp = ctx.enter_context(tc.tile_pool(name="vp", bufs=1))

        W_OF = {"q": (WQR, WQI), "k": (WKR, WKI), "v": (WVR, WVI)}

        for quad in range(2):           # 4 heads each
            with ExitStack() as qctx:
                mixp = qctx.enter_context(tc.tile_pool(name=f"mx{quad}", bufs=1))
                # QMIX per s-tile: [128, 4 heads x 192 (qr|qi|qrn)]
                qmix = [mixp.tile([128, 4 * 192], F32R, tag=f"qm{t}",
                                  name=f"qmix{quad}_{t}")
                        for t in range(ST)]
                kmix = [mixp.tile([128, 4 * 128], F32R, tag=f"km{t}",
                                  name=f"kmix{quad}_{t}")
                        for t in range(ST)]
                # VMIX [k-tile][128, 4h x 128]; even h4: [vr|vi], odd: [vi|vr]
                vmix = [vp.tile([128, 4 * 128], F32R, tag=f"vm{t}",
                                name=f"vmix{quad}_{t}")
                        for t in range(ST)]

                for shalf in range(2):
                    tiles = range(shalf * 4, shalf * 4 + 4)
                    with ExitStack() as pctx:
                        xp = pctx.enter_context(
                            tc.tile_pool(name=f"xs{quad}{shalf}", bufs=1))
                        wst = pctx.enter_context(
                            tc.tile_pool(name=f"ws{quad}{shalf}", bufs=2))
                        tmp = pctx.enter_context(
                            tc.tile_pool(name=f"tp{quad}{shalf}", bufs=2))
                        pps = pctx.enter_context(tc.tile_pool(
                            name=f"pp{quad}{shalf}", bufs=3, space="PSUM"))
                        trp = pctx.enter_context(tc.tile_pool(
                            name=f"tr{quad}{shalf}", bufs=3, space="PSUM"))

                        # x^T slices for these 4 s-tiles
                        xT = {}
                        for t in tiles:
                            xr_std = xp.tile([128, D], F32R, tag=f"xr{t}")
                            xi_std = xp.tile([128, D], F32R, tag=f"xi{t}")
                            nc.sync.dma_start(
                                xr_std[:], XR[t * 128:(t + 1) * 128, :])
                            nc.sync.dma_start(
                                xi_std[:], XI[t * 128:(t + 1) * 128, :])
                            xrT = xp.tile([128, D], F32R, tag=f"xrT{t}")
                            xiT = xp.tile([128, D], F32R, tag=f"xiT{t}")
                            for kc in range(KC):
                                ksl = slice(kc * 128, (kc + 1) * 128)
                                tp1 = trp.tile([128, 128], F32R, tag="tt")
                                nc.tensor.transpose(
                                    tp1[:], xr_std[:, ksl], ident[:])
                                nc.scalar.copy(xrT[:, ksl], tp1[:])
                                tp2 = trp.tile([128, 128], F32R, tag="tt")
                                nc.tensor.transpose(
                                    tp2[:], xi_std[:, ksl], ident[:])
                                nc.scalar.copy(xiT[:, ksl], tp2[:])
                            xT[t] = (xrT, xiT)

                        qsl = slice(quad * 256, (quad + 1) * 256)

                        for phase in ("q", "k", "v"):
                            Wr_d, Wi_d = W_OF[phase]
                            wr = wst.tile([128, KC * 256], F32R, tag="wr")
                            wi = wst.tile([128, KC * 256], F32R, tag="wi")
                            nc.sync.dma_start(
                                wr[:].rearrange("p (c n) -> p c n", c=KC),
                                Wr_d.rearrange("(c p) n -> p c n", p=128)
                                [:, :, qsl])
                            nc.sync.dma_start(
                                wi[:].rearrange("p (c n) -> p c n", c=KC),
                                Wi_d.rearrange("(c p) n -> p c n", p=128)
                                [:, :, qsl])

                            for t in tiles:
                                xrT, xiT = xT[t]
                                ps_r = pps.tile([128, 256], F32, tag="pj")
                                ps_i = pps.tile([128, 256], F32, tag="pj")
                                for kc in range(KC):
                                    ksl = slice(kc * 128, (kc + 1) * 128)
                                    nsl = slice(kc * 256, (kc + 1) * 256)
                                    nc.tensor.matmul(
                                        ps_r[:], xrT[:, ksl], wr[:, nsl],
                                        start=(kc == 0), stop=(kc == KC - 1))
                                for kc in range(KC):
                                    ksl = slice(kc * 128, (kc + 1) * 128)
                                    nsl = slice(kc * 256, (kc + 1) * 256)
                                    nc.tensor.matmul(
                                        ps_i[:], xiT[:, ksl], wi[:, nsl],
                                        start=(kc == 0), stop=(kc == KC - 1))

                                if phase in ("q", "k"):
                                    # rope in-psum (de-interleaved pairs)
                                    csl = ct[:, t * 64:(t + 1) * 64].rearrange(
                                        "p (h j) -> p h j", h=4)
                                    ssl = stb[:, t * 64:(t + 1) * 64].rearrange(
                                        "p (h j) -> p h j", h=4)
                                    for ps_t in (ps_r, ps_i):
                                        v3 = ps_t[:].rearrange(
                                            "p (h d) -> p h d", h=4)
                                        e = v3[:, :, 0:NJ]
                                        o = v3[:, :, NJ:2 * NJ]
                                        u1 = tmp.tile([128, 4, NJ], F32, tag="u1")
                                        u2 = tmp.tile([128, 4, NJ], F32, tag="u2")
                                        u3 = tmp.tile([128, 4, NJ], F32, tag="u3")
                                        u4 = tmp.tile([128, 4, NJ], F32, tag="u4")
                                        nc.vector.tensor_mul(u1[:], e, csl)
                                        nc.vector.tensor_mul(u2[:], o, ssl)
                                        nc.vector.tensor_mul(u3[:], o, csl)
                                        nc.vector.tensor_mul(u4[:], e, ssl)
                                        nc.vector.tensor_sub(e, u1[:], u2[:])
                                        nc.vector.tensor_add(o, u3[:], u4[:])

                                    # phase mix -> QMIX/KMIX
                                    tpc3 = tpc[:, qsl].rearrange(
                                        "p (h d) -> p h d", h=4)
                                    tps3 = tpsn[:, qsl].rearrange(
                                        "p (h d) -> p h d", h=4)
                                    r3 = ps_r[:].rearrange("p (h d) -> p h d", h=4)
                                    i3 = ps_i[:].rearrange("p (h d) -> p h d", h=4)
                                    if phase == "q":
                                        dst = qmix[t][:].rearrange(
                                            "p (h d) -> p h d", h=4)
                                        mixed_r = dst[:, :, 0:64]
                                        mixed_i = dst[:, :, 64:128]
                                    else:
                                        dst = kmix[t][:].rearrange(
                                            "p (h d) -> p h d", h=4)
                                        mixed_r = dst[:, :, 0:64]
                                        mixed_i = dst[:, :, 64:128]
                                    ua = tmp.tile([128, 4, 64], F32, tag="ma")
                                    ub = tmp.tile([128, 4, 64], F32, tag="mb")
                                    nc.vector.tensor_mul(ua[:], r3, tpc3)
                                    nc.vector.tensor_mul(ub[:], i3, tps3)
                                    nc.vector.tensor_sub(mixed_r, ua[:], ub[:])
                                    uc = tmp.tile([128, 4, 64], F32, tag="mc")
                                    ud = tmp.tile([128, 4, 64], F32, tag="md")
                                    nc.vector.tensor_mul(uc[:], r3, tps3)
                                    nc.vector.tensor_mul(ud[:], i3, tpc3)
                                    nc.vector.tensor_add(mixed_i, uc[:], ud[:])
                                    if phase == "q":
                                        nc.vector.tensor_scalar_mul(
                                            dst[:, :, 128:192], mixed_r, -1.0)
                                else:
                                    # v: parity-interleaved copies into VMIX
                                    vm = vmix[t][:].rearrange(
                                        "p (h x d) -> p h x d", h=2, x=2)
                                    r4 = ps_r[:].rearrange(
                                        "p (h e d) -> p h e d", h=2, e=2)
                                    i4 = ps_i[:].rearrange(
                                        "p (h e d) -> p h e d", h=2, e=2)
                                    # even local head (e=0): [vr|vi]
                                    nc.scalar.copy(vm[:, :, 0, 0:64].unsqueeze(2),
                                                   r4[:, :, 0:1, :])
                                    nc.scalar.copy(vm[:, :, 0, 64:128].unsqueeze(2),
                                                   i4[:, :, 0:1, :])
                                    # odd local head (e=1): [vi|vr]
                                    nc.scalar.copy(vm[:, :, 1, 0:64].unsqueeze(2),
                                                   i4[:, :, 1:2, :])
                                    nc.scalar.copy(vm[:, :, 1, 64:128].unsqueeze(2),
                                                   r4[:, :, 1:2, :])

                # ---- attention for this quad's 4 heads ----
                for h4 in range(4):
                    h = quad * 4 + h4
                    with ExitStack() as hctx:
                        hp = hctx.enter_context(
                            tc.tile_pool(name=f"hp{h}", bufs=1))
                        hw = hctx.enter_context(
                            tc.tile_pool(name=f"hw{h}", bufs=2))
                        accp = hctx.enter_context(
                            tc.tile_pool(name=f"ac{h}", bufs=4))
                        sps = hctx.enter_context(tc.tile_pool(
                            name=f"sp{h}", bufs=4, space="PSUM"))
                        ttp = hctx.enter_context(tc.tile_pool(
                            name=f"tq{h}", bufs=2, space="PSUM"))
                        avp = hctx.enter_context(tc.tile_pool(
                            name=f"avp{h}", bufs=2, space="PSUM"))

                        # A stack [128, S] from KMIX transposes
                        a_st = hp.tile([128, S], F32R)
                        for t in range(ST):
                            tp3 = ttp.tile([128, 128], F32R, tag="tt")
                            nc.tensor.transpose(
                                tp3[:], kmix[t][:, h4 * 128:(h4 + 1) * 128],
                                ident[:])
                            nc.scalar.copy(a_st[:, t * 128:(t + 1) * 128],
                                           tp3[:])

                        # p_T buffer [k-tile][128, 256] (2 q-tiles per chunk)
                        pt = hp.tile([128, ST * 256], F32R)

                        for t in range(ST):
                            tp4 = ttp.tile([128, 128], F32R, tag="tt")
                            nc.tensor.transpose(
                                tp4[:], qmix[t][:, h4 * 192:h4 * 192 + 128],
                                ident[:])
                            x_sl = hw.tile([128, 128], F32R, tag="xsl")
                            nc.scalar.copy(x_sl[:], tp4[:])
                            tp5 = ttp.tile([128, 128], F32R, tag="tt")
                            nc.tensor.transpose(
                                tp5[:],
                                qmix[t][:, h4 * 192 + 64:h4 * 192 + 192],
                                ident[:])
                            y_sl = hw.tile([128, 128], F32R, tag="ysl")
                            nc.scalar.copy(y_sl[:], tp5[:])

                            kwid = 128 * (t + 1)
                            pn = hw.tile([128, 1024], F32R, tag="pn")
                            accs = []
                            diag_m = None
                            for (pos, wdt, vw) in _chunks_for_qtile(t):
                                s_r = sps.tile([128, 512], F32, tag="sc")
                                s_i = sps.tile([128, 512], F32, tag="sc")
                                nc.tensor.matmul(
                                    s_r[:, 0:wdt], x_sl[:],
                                    a_st[:, pos:pos + wdt],
                                    start=True, stop=True)
                                nc.tensor.matmul(
                                    s_i[:, 0:wdt], y_sl[:],
                                    a_st[:, pos:pos + wdt],
                                    start=True, stop=True)
                                sq1 = hw.tile([128, 512], F32, tag="sq1")
                                sq2 = hw.tile([128, 512], F32, tag="sq2")
                                nc.scalar.activation(
                                    sq1[:, 0:vw], s_r[:, 0:vw], AF.Square)
                                nc.scalar.activation(
                                    sq2[:, 0:vw], s_i[:, 0:vw], AF.Square)
                                m_t = hw.tile([128, 512], F32, tag="m")
                                nc.gpsimd.tensor_add(
                                    m_t[:, 0:vw], sq1[:, 0:vw], sq2[:, 0:vw])
                                ln_t = hw.tile([128, 512], F32, tag="ln")
                                nc.scalar.activation(
                                    ln_t[:, 0:vw], m_t[:, 0:vw], AF.Ln,
                                    bias=epsc)
                                uu = hw.tile([128, 512], F32, tag="uu")
                                nc.scalar.activation(
                                    uu[:, 0:vw], ln_t[:, 0:vw], AF.Exp,
                                    scale=0.5, bias=lncc)
                                below = min(vw, kwid - 128 - pos)
                                if below > 0:
                                    ppb = hw.tile([128, 512], F32, tag="ppb")
                                    acc = accp.tile([128, 1], F32, tag="acc")
                                    nc.scalar.activation(
                                        ppb[:, 0:below], uu[:, 0:below],
                                        AF.Exp, accum_out=acc[:])
                                    accs.append(acc)
                                    nc.vector.tensor_copy(
                                        pn[:, pos:pos + below], ppb[:, 0:below])
                                if pos + vw == kwid:   # diag block
                                    ppd = hw.tile([128, 128], F32, tag="ppd")
                                    nc.scalar.activation(
                                        ppd[:], uu[:, below:below + 128],
                                        AF.Exp)
                                    diag_m = hw.tile([128, 128], F32, tag="dm")
                                    nc.vector.tensor_mul(
                                        diag_m[:], ppd[:], tri[:])
                                    dacc = accp.tile([128, 1], F32, tag="acc")
                                    nc.vector.reduce_sum(
                                        dacc[:], diag_m[:], axis=AX.X)
                                    accs.append(dacc)

                            dn = accp.tile([128, 1], F32, tag="dn")
                            if len(accs) == 1:
                                nc.vector.tensor_copy(dn[:], accs[0][:])
                            else:
                                nc.vector.tensor_add(
                                    dn[:], accs[0][:], accs[1][:])
                                for a_ in accs[2:]:
                                    nc.vector.tensor_add(dn[:], dn[:], a_[:])
                            rc = accp.tile([128, 1], F32, tag="rc")
                            nc.vector.reciprocal(rc[:], dn[:])
                            if kwid > 128:
                                nc.vector.tensor_scalar_mul(
                                    pn[:, 0:kwid - 128], pn[:, 0:kwid - 128],
                                    rc[:])
                            nc.vector.tensor_scalar_mul(
                                pn[:, kwid - 128:kwid], diag_m[:], rc[:])

                            qo = (t % 2) * 128
                            for kt in range(t + 1):
                                ptp = ttp.tile([128, 128], F32R, tag="tt")
                                nc.tensor.transpose(
                                    ptp[:], pn[:, kt * 128:(kt + 1) * 128],
                                    ident[:])
                                nc.scalar.copy(
                                    pt[:, kt * 256 + qo:kt * 256 + qo + 128],
                                    ptp[:])
                            if t % 2 == 1:
                                nc.vector.memset(
                                    pt[:, t * 256:t * 256 + 128].bitcast(F32),
                                    0.0)
                                qc = t // 2
                                av = avp.tile([128, 256], F32, tag="av")
                                for kt in range(t + 1):
                                    nc.tensor.matmul(
                                        av[:],
                                        vmix[kt][:, h4 * 128:(h4 + 1) * 128],
                                        pt[:, kt * 256:(kt + 1) * 256],
                                        start=(kt == 0), stop=(kt == t))
                                dch = h // 2
                                qq = slice(qc * 256, (qc + 1) * 256)
                                if h % 2 == 0:
                                    nc.scalar.copy(
                                        attn_r[dch][0:64, qq], av[0:64, :])
                                    nc.scalar.copy(
                                        attn_i[dch][64:128, qq], av[64:128, :])
                                else:
                                    nc.scalar.copy(
                                        attn_i[dch][0:64, qq], av[0:64, :])
                                    nc.scalar.copy(
                                        attn_r[dch][64:128, qq], av[64:128, :])

        # ---- out-projection ----
        with ExitStack() as octx:
            opw = octx.enter_context(tc.tile_pool(name="opw", bufs=1))
            opo = octx.enter_context(tc.tile_pool(name="opo", bufs=3))
            ops_ = octx.enter_context(
                tc.tile_pool(name="ops", bufs=2, space="PSUM"))
            wor = opw.tile([128, 4 * D], F32R)
            woi = opw.tile([128, 4 * D], F32R)
            nc.sync.dma_start(
                wor[:].rearrange("p (c n) -> p c n", c=4),
                WOR.rearrange("(c p) n -> p c n", p=128))
            nc.sync.dma_start(
                woi[:].rearrange("p (c n) -> p c n", c=4),
                WOI.rearrange("(c p) n -> p c n", p=128))
            for t in range(ST):
                ssl = slice(t * 128, (t + 1) * 128)
                for dchunk in range(2):
                    dsl = slice(dchunk * 512, (dchunk + 1) * 512)
                    pr = ops_.tile([128, 512], F32, tag="op")
                    for kcc in range(4):
                        nc.tensor.matmul(
                            pr[:], attn_r[kcc][:, ssl],
                            wor[:, kcc * D + dchunk * 512:
                                kcc * D + dchunk * 512 + 512],
                            start=(kcc == 0), stop=(kcc == 3))
                    orr = opo.tile([128, 512], F32, tag="oo")
                    nc.scalar.copy(orr[:], pr[:])
                    nc.sync.dma_start(OUTR[ssl, dsl], orr[:])
                    pi = ops_.tile([128, 512], F32, tag="op")
                    for kcc in range(4):
                        nc.tensor.matmul(
                            pi[:], attn_i[kcc][:, ssl],
                            woi[:, kcc * D + dchunk * 512:
                                kcc * D + dchunk * 512 + 512],
                            start=(kcc == 0), stop=(kcc == 3))
                    oii = opo.tile([128, 512], F32, tag="oo")
                    nc.scalar.copy(oii[:], pi[:])
                    nc.sync.dma_start(OUTI[ssl, dsl], oii[:])

    nc.compile()
    return nc


_PROGRAM = None


def _get_program():
    global _PROGRAM
    if _PROGRAM is None:
        _PROGRAM = build_program()
    return _PROGRAM


def _host_prep(inputs):
    real = np.asarray(inputs['real'], np.float32)
    imag = np.asarray(inputs['imag'], np.float32)
    ent = np.asarray(inputs['entanglement'], np.float64)
    phase = np.asarray(inputs['phase_shifts'], np.float64)
    freqs = np.asarray(inputs['rotary_freqs'], np.float64)
    strength = float(np.asarray(inputs['interference_strength']).reshape(-1)[0])
    temp = float(np.asarray(inputs['attention_temperature']).reshape(-1)[0])

    # per-head dim permutation: j<16 -> 2j ; 16<=j<32 -> 2(j-16)+1 ; else j
    p64 = np.empty(HD, np.int64)
    p64[0:NJ] = np.arange(NJ) * 2
    p64[NJ:2 * NJ] = np.arange(NJ) * 2 + 1
    p64[2 * NJ:] = np.arange(2 * NJ, HD)

    def prep_qk(Wname, scaled):
        W = np.asarray(inputs[Wname], np.float64).reshape(D, H, HD)
        W = np.einsum('khd,hx->kxd', W, ent)
        W = W[:, :, p64]
        if scaled:
            W = W * 0.125
        return W

    wq_r3 = prep_qk('wq_r', True)
    wq_i3 = prep_qk('wq_i', True)
    wk_r3 = prep_qk('wk_r', False)
    wk_i3 = prep_qk('wk_i', False)
    wv_r3 = np.asarray(inputs['wv_r'], np.float64).reshape(D, H, HD)
    wv_i3 = np.asarray(inputs['wv_i'], np.float64).reshape(D, H, HD)
    wo_r = np.asarray(inputs['wo_r'], np.float64)
    wo_i = np.asarray(inputs['wo_i'], np.float64)

    c = 1.0 / (1.0 + np.exp(-strength)) / max(temp, 0.01)

    pcs = np.cos(phase)[:, p64]
    pss = np.sin(phase)[:, p64]

    # rope tables [128, (t, h4, j)]
    s_idx = np.arange(S).reshape(ST, 128)
    theta = s_idx[:, :, None] * freqs[None, None, :]        # [ST, 128, NJ]
    ct_h = np.cos(theta)[:, :, None, :]                     # [ST,128,1,NJ]
    st_h = np.sin(theta)[:, :, None, :]
    ct_h = np.broadcast_to(ct_h, (ST, 128, 4, NJ))
    st_h = np.broadcast_to(st_h, (ST, 128, 4, NJ))
    ct_h = ct_h.transpose(1, 0, 2, 3).reshape(128, ST * 4 * NJ).astype(np.float32)
    st_h = st_h.transpose(1, 0, 2, 3).reshape(128, ST * 4 * NJ).astype(np.float32)

    tri = (np.arange(128)[None, :] <= np.arange(128)[:, None]).astype(np.float32)

    cc = np.zeros((128, 2), np.float32)
    cc[:, 0] = 1e-6
    cc[:, 1] = np.log(c)

    idn = np.eye(128, dtype=np.float32)

    # WOI row permutation: per pair, odd head first (see attn_i layout)
    woi_perm = np.arange(H * HD).reshape(H // 2, 2, HD)[:, ::-1, :].reshape(-1)

    in_maps = []
    for core in range(N_CORES):
        b = core >> 1
        g = core & 1
        hs = slice(g * HPC, (g + 1) * HPC)
        woi_g = wo_i[g * HPC * HD:(g + 1) * HPC * HD]
        woi_g = woi_g[np.arange(HPC * HD).reshape(HPC // 2, 2, HD)
                      [:, ::-1, :].reshape(-1)]
        m = {
            'XR': round_f32r(real[b]),
            'XI': round_f32r(imag[b]),
            'WQR': round_f32r(wq_r3[:, hs].reshape(D, HPC * HD)),
            'WQI': round_f32r(wq_i3[:, hs].reshape(D, HPC * HD)),
            'WKR': round_f32r(wk_r3[:, hs].reshape(D, HPC * HD)),
            'WKI': round_f32r(wk_i3[:, hs].reshape(D, HPC * HD)),
            'WVR': round_f32r(wv_r3[:, hs].reshape(D, HPC * HD)),
            'WVI': round_f32r(wv_i3[:, hs].reshape(D, HPC * HD)),
            'WOR': round_f32r(wo_r[g * HPC * HD:(g + 1) * HPC * HD]),
            'WOI': round_f32r(woi_g),
            'IDN': idn,
            'CT': ct_h, 'STB': st_h,
            'TPC': round_f32r(np.broadcast_to(
                pcs[hs].reshape(1, HPC * HD), (128, HPC * HD)).copy()),
            'TPS': round_f32r(np.broadcast_to(
                pss[hs].reshape(1, HPC * HD), (128, HPC * HD)).copy()),
            'TRI': tri, 'CC': cc,
        }
        in_maps.append(m)
    return in_maps


def _fallback(inputs):
    """Exact numpy fallback for inputs the fast path doesn't support
    (nonzero attention_mask or q/k/v biases — never produced by the
    standard setup_inputs)."""
    import math
    real = np.asarray(inputs['real'], np.float64)
    imag = np.asarray(inputs['imag'], np.float64)
    b, s, d = real.shape
    phase = np.asarray(inputs['phase_shifts'], np.float64)
    h, hd = phase.shape

    def proj(x, w, bias):
        return (x @ np.asarray(w, np.float64)
                + np.asarray(bias, np.float64)).reshape(
                    b, s, h, hd).transpose(0, 2, 1, 3)

    q_r = proj(real, inputs['wq_r'], inputs['bq_r'])
    k_r = proj(real, inputs['wk_r'], inputs['bk_r'])
    v_r = proj(real, inputs['wv_r'], inputs['bv_r'])
    q_i = proj(imag, inputs['wq_i'], inputs['bq_i'])
    k_i = proj(imag, inputs['wk_i'], inputs['bk_i'])
    v_i = proj(imag, inputs['wv_i'], inputs['bv_i'])

    freqs = np.asarray(inputs['rotary_freqs'], np.float64)
    rd = 2 * freqs.shape[0]
    pos = np.arange(s)
    emb = pos[:, None] * freqs[None, :]
    cos = np.cos(emb)[None, None]
    sin = np.sin(emb)[None, None]

    def rot(x):
        xr, xp = x[..., :rd], x[..., rd:]
        xr = xr.reshape(*xr.shape[:-1], rd // 2, 2)
        x0 = xr[..., 0] * cos - xr[..., 1] * sin
        x1 = xr[..., 1] * cos + xr[..., 0] * sin
        xr = np.stack([x0, x1], axis=-1).reshape(*x.shape[:-1], rd)
        return np.concatenate([xr, xp], axis=-1)

    q_r, k_r = rot(q_r), rot(k_r)
    q_i, k_i = rot(q_i), rot(k_i)
    ent = np.asarray(inputs['entanglement'], np.float64)
    q_r = np.einsum('bhsd,hx->bxsd', q_r, ent)
    q_i = np.einsum('bhsd,hx->bxsd', q_i, ent)
    k_r = np.einsum('bhsd,hx->bxsd', k_r, ent)
    k_i = np.einsum('bhsd,hx->bxsd', k_i, ent)
    pc = np.cos(phase)[None, :, None, :]
    ps = np.sin(phase)[None, :, None, :]
    qr, qi = q_r * pc - q_i * ps, q_r * ps + q_i * pc
    kr, ki = k_r * pc - k_i * ps, k_r * ps + k_i * pc
    scale = 1.0 / math.sqrt(hd)
    ar = (np.einsum('bhqd,bhkd->bhqk', qr, kr)
          + np.einsum('bhqd,bhkd->bhqk', qi, ki)) * scale
    ai = (np.einsum('bhqd,bhkd->bhqk', qi, kr)
          - np.einsum('bhqd,bhkd->bhqk', qr, ki)) * scale
    mag = np.sqrt(ar ** 2 + ai ** 2 + 1e-6)
    causal = np.triu(np.ones((s, s), bool), 1)[None, None]
    amask = np.asarray(inputs['attention_mask'], bool)
    fm = causal | amask[:, None, None, :]
    strength = float(np.asarray(inputs['interference_strength']).reshape(-1)[0])
    temp = float(np.asarray(inputs['attention_temperature']).reshape(-1)[0])
    cs = (1.0 / (1.0 + np.exp(-strength))) / max(temp, 0.01)
    logits = np.where(fm, -np.inf, mag * cs)
    logits = logits - logits.max(-1, keepdims=True)
    w = np.exp(logits)
    w = w / w.sum(-1, keepdims=True)
    out_r = np.einsum('bhqk,bhkd->bhqd', w, v_r).transpose(
        0, 2, 1, 3).reshape(b, s, d)
    out_i = np.einsum('bhqk,bhkd->bhqd', w, v_i).transpose(
        0, 2, 1, 3).reshape(b, s, d)
    out_r = out_r @ np.asarray(inputs['wo_r'], np.float64) \
        + np.asarray(inputs['bo_r'], np.float64)
    out_i = out_i @ np.asarray(inputs['wo_i'], np.float64) \
        + np.asarray(inputs['bo_i'], np.float64)
    return out_r.astype(np.float32), out_i.astype(np.float32)


def kernel(**inputs):
    needs_fallback = (
        np.any(np.asarray(inputs['attention_mask']))
        or any(np.any(np.asarray(inputs[k]))
               for k in ('bq_r', 'bk_r', 'bv_r', 'bq_i', 'bk_i', 'bv_i'))
    )
    if needs_fallback:
        return _fallback(inputs)

    nc = _get_program()
    in_maps = _host_prep(inputs)
    res = run_bass_kernel_spmd(nc, in_maps, list(range(N_CORES)))

    bo_r = np.asarray(inputs['bo_r'], np.float32)
    bo_i = np.asarray(inputs['bo_i'], np.float32)
    out_r = np.empty((B, S, D), np.float32)
    out_i = np.empty((B, S, D), np.float32)
    for b in range(B):
        out_r[b] = (res.results[2 * b]['OUTR']
                    + res.results[2 * b + 1]['OUTR'] + bo_r)
        out_i[b] = (res.results[2 * b]['OUTI']
                    + res.results[2 * b + 1]['OUTI'] + bo_i)
    return out_r, out_i


if __name__ == "__main__":
    _get_program()
    print("program built OK")
